# revision 9
# baseline (speedup 1.0000x reference)
"""Trainium2 Bass kernel for GQA attention (B=2, S=2048, H=2048, 32 Q heads,
8 KV heads, HD=64, RoPE, causal) with output projection.

Sharding: TP=4 over heads within each batch, DP=2 over batch -> 8 cores.
Core c handles batch c//4, head-rank c%4 (8 Q heads, 2 KV heads).
Each core computes a partial o_proj output [S, H]; the host sums the 4
partials per batch (cheaper than on-device all-reduce at these sizes).

Per-core layout (host produces bf16 pre-cast, transposed views):
  xt  [H, S]   = hidden[b].T                  bf16
  wqt [H, 512] = Wq[head-block r].T           bf16  (head order h0,h4,h1,h5,h2,h6,h3,h7)
  wkt [H, 128] = Wk[kv-block r].T             bf16
  wvt [H, 128] = Wv[kv-block r].T             bf16
  wot [512, H] = Wo[:, block r].T             bf16  (rows in same head order as wqt)
  c2/ss [128, S] RoPE cos/sin tables bf16 (two 64-row head blocks stacked)
  rot [128, 128] rotate-half permutation (+/-1) bf16, msk [128,128] causal bias f32

On device: direct DMA into persistent bf16 tiles (no staging casts), weight
and x chunk-0 loads interleaved so Q-proj matmuls start within a few us.
Q^T/K^T/V projections on PE, RoPE via PE rotation matmul + DVE, scores^T =
K^T.T Q^T per head (causal-trimmed), exp on ACT as ONE instruction per score
tile (strided 3D AP over both head halves on diagonal tiles; ACT stream is
pure Exp -> a single table load for the whole kernel), AV with
ones-augmented V (causal-trimmed on diagonal tiles) to get softmax
denominators for free, reciprocal on DVE (reciprocal_approx_fast, no ACT
table switches), normalize, o_proj. fp32 PSUM throughout.
"""

import numpy as np
from contextlib import ExitStack

import concourse.bass as bass
import concourse.bacc as bacc
import concourse.mybir as mybir
import concourse.tile as tile
from concourse.bass_utils import run_bass_kernel_spmd

F32 = mybir.dt.float32
BF16 = mybir.dt.bfloat16
AF = mybir.ActivationFunctionType

B, S, H = 2, 2048, 2048
NH, NKV, HD = 32, 8, 64
TP = 4                      # head-parallel ranks per batch
NQO = NH // TP * HD         # 512 per-core q features (8 heads)
NKO = NKV // TP * HD        # 128 per-core kv features (2 heads)
NHL = NH // TP              # 8 local q heads
EXP_SCALE = 1.0 / 8.0       # 1/sqrt(HD)
MASK_VAL = -30000.0
P = 128
QC = 512                    # q-chunk (one PSUM bank of fp32)
NSC = S // QC               # 4 q/s chunks
NPT = S // P                # 16 partition tiles of S
NHT = H // P                # 16 partition tiles of H


def build_nc():
    nc = bacc.Bacc("TRN2", target_bir_lowering=False, debug=False, num_devices=8)

    xt = nc.dram_tensor("xt", [H, S], BF16, kind="ExternalInput").ap()
    wqt = nc.dram_tensor("wqt", [H, NQO], BF16, kind="ExternalInput").ap()
    wkt = nc.dram_tensor("wkt", [H, NKO], BF16, kind="ExternalInput").ap()
    wvt = nc.dram_tensor("wvt", [H, NKO], BF16, kind="ExternalInput").ap()
    wot = nc.dram_tensor("wot", [NQO, H], BF16, kind="ExternalInput").ap()
    c2 = nc.dram_tensor("c2", [P, S], BF16, kind="ExternalInput").ap()
    ss = nc.dram_tensor("ss", [P, S], BF16, kind="ExternalInput").ap()
    msk = nc.dram_tensor("msk", [P, P], F32, kind="ExternalInput").ap()
    rot = nc.dram_tensor("rot", [P, P], BF16, kind="ExternalInput").ap()
    y = nc.dram_tensor("y", [S, H], F32, kind="ExternalOutput").ap()

    xt_t = xt.rearrange("(n p) s -> n p s", p=P)
    wqt_t = wqt.rearrange("(n p) o -> n p o", p=P)
    wkt_t = wkt.rearrange("(n p) o -> n p o", p=P)
    wvt_t = wvt.rearrange("(n p) o -> n p o", p=P)
    wot_t = wot.rearrange("(n p) o -> n p o", p=P)
    y_t = y.rearrange("(n p) o -> n p o", p=P)

    with tile.TileContext(nc) as tc, ExitStack() as ctx:
        persist = ctx.enter_context(tc.tile_pool(name="persist", bufs=1))
        xp = ctx.enter_context(tc.tile_pool(name="xp", bufs=32))
        p2 = ctx.enter_context(tc.tile_pool(name="p2", bufs=7))
        p2a = ctx.enter_context(tc.tile_pool(name="p2a", bufs=3))
        p3 = ctx.enter_context(tc.tile_pool(name="p3", bufs=4))
        # all 8 PSUM banks shared across projection + attention + o_proj:
        #   tag "sp"  [128, 1024] x2 (4 banks): score pairs, o_proj, Q/K proj
        #   tag "aux" [128, 512]  x4 (4 banks): AV accum, recip bcast, rope, V
        psum = ctx.enter_context(tc.tile_pool(name="psum", bufs=2, space="PSUM"))

        c2_sb = persist.tile([P, S], BF16, tag="c2", name="c2sb")
        ss_sb = persist.tile([P, S], BF16, tag="ss", name="sssb")
        msk_sb = persist.tile([P, P], F32, tag="msk", name="msksb")
        rot_sb = persist.tile([P, P], BF16, tag="rot", name="rotsb")
        ones65b = persist.tile([65, 64], BF16, tag="ones65b", name="ones65b")

        qtbc = [[persist.tile([P, QC], BF16, tag=f"qtbc{t}_{sc}", name=f"qtbc{t}_{sc}")
                 for sc in range(NSC)] for t in range(4)]
        ktbc = [persist.tile([P, QC], BF16, tag=f"ktbc{sc}", name=f"ktbc{sc}")
                for sc in range(NSC)]
        vaug = [persist.tile([P, 130], BF16, tag=f"vaug{i}", name=f"vaug{i}")
                for i in range(NPT)]
        atbc = [[persist.tile([P, QC], BF16, tag=f"atbc{t}_{qc}", name=f"atbc{t}_{qc}")
                 for qc in range(NSC)] for t in range(4)]
        wotb = [persist.tile([P, S], BF16, tag=f"wotb{t}", name=f"wotb{t}") for t in range(4)]
        wqtb = [persist.tile([P, NQO], BF16, tag=f"wqtb{i}", name=f"wqtb{i}") for i in range(NHT)]
        wktb = [persist.tile([P, NKO], BF16, tag=f"wktb{i}", name=f"wktb{i}") for i in range(NHT)]
        wvtb = [persist.tile([P, NKO], BF16, tag=f"wvtb{i}", name=f"wvtb{i}") for i in range(NHT)]

        # ---- startup: weight + x chunk-0 loads interleaved so the chunk-0
        # Q-proj accumulation (i = 0..15) can start as soon as tile 0 lands.
        xtbc = [[None] * NSC for _ in range(NHT)]
        for i in range(NHT):
            e1 = nc.sync if i % 2 == 0 else nc.gpsimd
            e2 = nc.gpsimd if i % 2 == 0 else nc.sync
            e1.dma_start(wqtb[i][:], wqt_t[i])
            e2.dma_start(wktb[i][:], wkt_t[i])
            e1.dma_start(wvtb[i][:], wvt_t[i])
            xb = xp.tile([P, QC], BF16, tag="xtbc", name=f"xtbc{i}_0")
            e2.dma_start(xb[:], xt_t[i][:, 0:QC])
            xtbc[i][0] = xb
        nc.sync.dma_start(rot_sb[:], rot[:])
        nc.gpsimd.dma_start(msk_sb[:], msk[:])
        nc.sync.dma_start(c2_sb[:], c2[:])
        nc.gpsimd.dma_start(ss_sb[:], ss[:])
        nc.gpsimd.memset(ones65b[64:65, :], 1.0)
        for t in range(4):
            deng = nc.sync if t % 2 == 0 else nc.gpsimd
            deng.dma_start(wotb[t][:], wot_t[t])

        def rope_tile(dst_ap, ps, sc):
            """RoPE: dst = raw*C2 + (R @ raw)*SS for one [128, 512] chunk."""
            ssl = slice(QC * sc, QC * (sc + 1))
            raw = p2a.tile([P, QC], BF16, tag="rope_raw")
            nc.scalar.copy(raw[:], ps[:])
            rps = psum.tile([P, QC], F32, tag="aux", bufs=4, name="rps")
            nc.tensor.matmul(rps[:], lhsT=rot_sb[:], rhs=raw[:],
                             start=True, stop=True)
            t1 = p2a.tile([P, QC], F32, tag="rope_t1")
            nc.vector.tensor_mul(t1[:], raw[:], c2_sb[:, ssl])
            t2 = p2a.tile([P, QC], F32, tag="rope_t2")
            nc.vector.tensor_mul(t2[:], rps[:], ss_sb[:, ssl])
            nc.vector.tensor_add(dst_ap, t1[:], t2[:])

        def attn_step(hp, qc, ki, avpA, avpB, last):
            """Both heads of a pair share one 2-bank score tile; exp always
            runs as ONE ACT instruction (strided 3D AP on diagonal tiles)."""
            j = ki - 4 * qc
            col0 = P * j if j >= 0 else 0
            kc = P * (ki % 4)
            sp = psum.tile([P, 2 * QC], F32, tag="sp", bufs=2, name="sp")
            nc.tensor.matmul(
                sp[:, col0:QC],
                lhsT=ktbc[ki // 4][0:64, kc:kc + P],
                rhs=qtbc[hp][qc][0:64, col0:QC],
                start=True, stop=True,
            )
            nc.tensor.matmul(
                sp[:, QC + col0:2 * QC],
                lhsT=ktbc[ki // 4][64:128, kc:kc + P],
                rhs=qtbc[hp][qc][64:128, col0:QC],
                start=True, stop=True,
            )
            ep = p2.tile([P, 2 * QC], BF16, tag="ep")
            if j >= 0:
                nc.vector.tensor_add(sp[:, col0:col0 + P],
                                     sp[:, col0:col0 + P], msk_sb[:])
                nc.vector.tensor_add(sp[:, QC + col0:QC + col0 + P],
                                     sp[:, QC + col0:QC + col0 + P], msk_sb[:])
                nc.scalar.activation(ep[:, col0:QC], sp[:, col0:QC],
                                     AF.Exp, scale=EXP_SCALE)
                nc.scalar.activation(ep[:, QC + col0:2 * QC],
                                     sp[:, QC + col0:2 * QC],
                                     AF.Exp, scale=EXP_SCALE)
            else:
                nc.scalar.activation(ep[:], sp[:], AF.Exp, scale=EXP_SCALE)
            nc.tensor.matmul(
                avpA[:, col0:QC], lhsT=vaug[ki][:, 0:65],
                rhs=ep[:, col0:QC],
                start=(ki == 0), stop=last,
            )
            nc.tensor.matmul(
                avpB[:, col0:QC], lhsT=vaug[ki][:, 65:130],
                rhs=ep[:, QC + col0:2 * QC],
                start=(ki == 0), stop=last,
            )

        def normalize(hp, off, qc, avp):
            # 1/rowsum on DVE (reciprocal_approx_fast, ~18 correct bits) so
            # the ACT engine's instruction stream stays pure Exp (no table
            # switches); broadcast across partitions via a K=1 matmul.
            atrs = p2a.tile([65, QC], F32, tag="atrs")
            nc.vector.tensor_copy(atrs[:], avp[0:65, :])
            rcp = p2a.tile([65, QC], BF16, tag="rcp")
            with nc.allow_low_precision(reason="softmax denom bcast in bf16"):
                nc.vector.reciprocal(rcp[64:65, :], atrs[64:65, :])
            rbc = psum.tile([64, QC], F32, tag="aux", bufs=4, name="rbc")
            nc.tensor.matmul(rbc[:], lhsT=ones65b[64:65, 0:64],
                             rhs=rcp[64:65, :], start=True, stop=True)
            nc.vector.tensor_mul(atbc[hp][qc][off:off + 64, :],
                                 atrs[0:64, :], rbc[:])

        def oproj_piece(qc, st):
            stj = st - 4 * qc
            for oc in range(NSC):
                op = psum.tile([P, QC], F32, tag="sp", bufs=2, name="op")
                for ft in range(4):
                    nc.tensor.matmul(
                        op[:],
                        lhsT=atbc[ft][qc][:, P * stj:P * (stj + 1)],
                        rhs=wotb[ft][:, QC * oc:QC * (oc + 1)],
                        start=(ft == 0), stop=(ft == 3),
                    )
                ost = p3.tile([P, QC], F32, tag="ost")
                nc.vector.tensor_copy(ost[:], op[:])
                nc.sync.dma_start(y_t[st][:, QC * oc:QC * (oc + 1)], ost[:])

        from collections import deque
        oproj_q = deque()
        pending_norms = []
        for sc in range(NSC):
            # Q^T chunks
            for t in range(4):
                ps = psum.tile([P, QC], F32, tag="sp", bufs=2, name="qkps")
                for i in range(NHT):
                    nc.tensor.matmul(
                        ps[:], lhsT=wqtb[i][:, P * t:P * (t + 1)],
                        rhs=xtbc[i][sc][:],
                        start=(i == 0), stop=(i == NHT - 1),
                    )
                rope_tile(qtbc[t][sc][:], ps, sc)
            # K^T chunk
            ps = psum.tile([P, QC], F32, tag="sp", bufs=2, name="qkps")
            for i in range(NHT):
                nc.tensor.matmul(
                    ps[:], lhsT=wktb[i][:], rhs=xtbc[i][sc][:],
                    start=(i == 0), stop=(i == NHT - 1),
                )
            rope_tile(ktbc[sc][:], ps, sc)
            # V tiles in this chunk
            for j in range(4 * sc, 4 * sc + 4):
                jj = j - 4 * sc
                ps = psum.tile([P, NKO], F32, tag="aux", bufs=4, name="vps")
                for i in range(NHT):
                    nc.tensor.matmul(
                        ps[:], lhsT=xtbc[i][sc][:, P * jj:P * (jj + 1)],
                        rhs=wvtb[i][:],
                        start=(i == 0), stop=(i == NHT - 1),
                    )
                nc.vector.tensor_copy(vaug[j][:, 0:64], ps[:, 0:64])
                nc.vector.tensor_copy(vaug[j][:, 65:129], ps[:, 64:128])
                nc.gpsimd.memset(vaug[j][:, 64:65], 1.0)
                nc.gpsimd.memset(vaug[j][:, 129:130], 1.0)

            # prefetch next chunk's X columns (overlaps with attention)
            if sc + 1 < NSC:
                for i in range(NHT):
                    deng = nc.sync if i % 2 == 0 else nc.gpsimd
                    xb = xp.tile([P, QC], BF16, tag="xtbc",
                                 name=f"xtbc{i}_{sc + 1}")
                    deng.dma_start(xb[:], xt_t[i][:, QC * (sc + 1):QC * (sc + 2)])
                    xtbc[i][sc + 1] = xb

            # ---- attention for qc = sc (causal: only needs chunks <= sc) --
            qc = sc
            nkt = 4 * qc + 4
            for hp in range(4):
                avpA = psum.tile([65, QC], F32, tag="aux", bufs=4, name="avpA")
                avpB = psum.tile([65, QC], F32, tag="aux", bufs=4, name="avpB")
                for ki in range(nkt):
                    attn_step(hp, qc, ki, avpA, avpB, ki == nkt - 1)
                # deferred work from the previous group fills PE while this
                # group's exps/AVs drain
                prev, pending_norms = pending_norms, [
                    (hp, 0, qc, avpA), (hp, 64, qc, avpB)]
                for args in prev:
                    normalize(*args)
                if oproj_q:
                    oproj_piece(*oproj_q.popleft())
            for st in range(4 * qc, 4 * qc + 4):
                oproj_q.append((qc, st))
        for args in pending_norms:
            normalize(*args)
        while oproj_q:
            oproj_piece(*oproj_q.popleft())

    nc.compile()
    return nc


def _host_tables():
    import ml_dtypes
    BF = ml_dtypes.bfloat16
    inv_freq = 1.0 / (10000.0 ** (np.arange(0, HD, 2, dtype=np.float32) / HD))
    pos = np.arange(S, dtype=np.float32)
    freqs = np.einsum('s,d->sd', pos, inv_freq)          # [S, 32]
    emb = np.concatenate([freqs, freqs], axis=-1)        # [S, 64]
    cosT = np.cos(emb).T.astype(np.float32)              # [64, S]
    sinT = np.sin(emb).T.astype(np.float32)
    c2 = np.ascontiguousarray(np.vstack([cosT, cosT])).astype(BF)   # [128, S]
    # sign of rotate_half is encoded in the rot matrix below; ss is plain sin
    ss = np.ascontiguousarray(np.vstack([sinT, sinT])).astype(BF)
    # rotate-half as a matmul: out[d] = sum_d' R[d', d] * in[d']
    R64 = np.zeros((HD, HD), dtype=np.float32)
    for d in range(32):
        R64[d + 32, d] = -1.0       # out[d] = -in[d+32]
        R64[d, d + 32] = 1.0        # out[d+32] = in[d]
    rot = np.zeros((P, P), dtype=np.float32)
    rot[0:64, 0:64] = R64
    rot[64:128, 64:128] = R64
    # causal bias for a diagonal 128x128 tile in scores^T[k, q] layout
    kk = np.arange(P)[:, None]
    qq = np.arange(P)[None, :]
    msk = np.where(kk <= qq, 0.0, MASK_VAL).astype(np.float32)
    rot = rot.astype(BF)   # exact: entries are 0/+-1
    return c2, ss, rot, msk


# q/o head order within a rank block: pair heads (u, u+4) in each 128-row tile
_HEAD_ORDER = [0, 4, 1, 5, 2, 6, 3, 7]


def _make_in_maps(hidden_states, Wq, Wk, Wv, Wo):
    import ml_dtypes
    BF = ml_dtypes.bfloat16
    hs = np.asarray(hidden_states, dtype=np.float32)
    Wq = np.asarray(Wq, dtype=np.float32)
    Wk = np.asarray(Wk, dtype=np.float32)
    Wv = np.asarray(Wv, dtype=np.float32)
    Wo = np.asarray(Wo, dtype=np.float32)
    c2, ss, rot, msk = _host_tables()
    in_maps = []
    for c in range(8):
        b, r = c // 4, c % 4
        # row indices of Wq (= cols of Wo) for this rank, in device head order
        qrows = np.concatenate([
            np.arange(HD) + (NHL * r + u) * HD for u in _HEAD_ORDER
        ])
        in_maps.append({
            "xt": np.ascontiguousarray(hs[b].T).astype(BF),
            "wqt": np.ascontiguousarray(Wq[qrows, :].T).astype(BF),
            "wkt": np.ascontiguousarray(Wk[NKO * r:NKO * (r + 1), :].T).astype(BF),
            "wvt": np.ascontiguousarray(Wv[NKO * r:NKO * (r + 1), :].T).astype(BF),
            "wot": np.ascontiguousarray(Wo[:, qrows].T).astype(BF),
            "c2": c2, "ss": ss, "msk": msk, "rot": rot,
        })
    return in_maps


_NC = None


def _get_nc():
    global _NC
    if _NC is None:
        _NC = build_nc()
    return _NC


def run_cores(hidden_states, Wq, Wk, Wv, Wo, **run_kwargs):
    """Run the SPMD kernel; returns (out [B,S,H] fp32, BassKernelResults)."""
    nc = _get_nc()
    in_maps = _make_in_maps(hidden_states, Wq, Wk, Wv, Wo)
    res = run_bass_kernel_spmd(nc, in_maps, list(range(8)), **run_kwargs)
    out = np.zeros((B, S, H), dtype=np.float32)
    for c in range(8):
        out[c // 4] += res.results[c]["y"]
    return out, res


def kernel(hidden_states, Wq, Wk, Wv, Wo):
    out, _ = run_cores(hidden_states, Wq, Wk, Wv, Wo)
    return out


# revision 18
# speedup vs baseline: 1.1112x; 1.1112x over previous
"""Trainium2 Bass kernel for GQA attention (B=2, S=2048, H=2048, 32 Q heads,
8 KV heads, HD=64, RoPE, causal) with output projection.

Sharding: TP=4 over heads within each batch, DP=2 over batch -> 8 cores.
Core c handles batch c//4, head-rank c%4 (8 Q heads, 2 KV heads).
Each core computes a partial o_proj output [S, H]; the host sums the 4
partials per batch (cheaper than on-device all-reduce at these sizes).

Per-core layout (host produces bf16 pre-cast, transposed views):
  xt  [H, S]   = hidden[b].T                  bf16
  wqt [H, 512] = Wq[head-block r].T           bf16  (head order h0,h4,h1,h5,h2,h6,h3,h7)
  wkt [H, 128] = Wk[kv-block r].T             bf16
  wvt [H, 128] = Wv[kv-block r].T             bf16
  wot [512, H] = Wo[:, block r].T             bf16  (rows in same head order as wqt)
  c2/ss [128, S] RoPE cos/sin tables bf16 (two 64-row head blocks stacked)
  rot [128, 128] rotate-half permutation (+/-1) bf16, msk [128,128] causal bias f32

On device: direct DMA into persistent bf16 tiles (no staging casts), weight
and x chunk-0 loads interleaved so Q-proj matmuls start within a few us.
Q^T/K^T/V projections on PE, RoPE via PE rotation matmul + DVE, scores^T =
K^T.T Q^T per head (causal-trimmed), exp on ACT as ONE instruction per score
tile (strided 3D AP over both head halves on diagonal tiles; ACT stream is
pure Exp -> a single table load for the whole kernel), AV with
ones-augmented V (causal-trimmed on diagonal tiles) to get softmax
denominators for free, reciprocal on DVE (reciprocal_approx_fast, no ACT
table switches), normalize, o_proj. fp32 PSUM throughout.
"""

import numpy as np
from contextlib import ExitStack

import concourse.bass as bass
import concourse.bacc as bacc
import concourse.mybir as mybir
import concourse.tile as tile
from concourse.bass_utils import run_bass_kernel_spmd

F32 = mybir.dt.float32
BF16 = mybir.dt.bfloat16
AF = mybir.ActivationFunctionType

B, S, H = 2, 2048, 2048
NH, NKV, HD = 32, 8, 64
TP = 4                      # head-parallel ranks per batch
NQO = NH // TP * HD         # 512 per-core q features (8 heads)
NKO = NKV // TP * HD        # 128 per-core kv features (2 heads)
NHL = NH // TP              # 8 local q heads
EXP_SCALE = 1.0 / 8.0       # 1/sqrt(HD)
MASK_VAL = -30000.0
P = 128
QC = 512                    # q-chunk (one PSUM bank of fp32)
NSC = S // QC               # 4 q/s chunks
NPT = S // P                # 16 partition tiles of S
NHT = H // P                # 16 partition tiles of H


def build_nc():
    nc = bacc.Bacc("TRN2", target_bir_lowering=False, debug=False, num_devices=8)

    xt = nc.dram_tensor("xt", [H, S], BF16, kind="ExternalInput").ap()
    wqt = nc.dram_tensor("wqt", [H, NQO], BF16, kind="ExternalInput").ap()
    wkt = nc.dram_tensor("wkt", [H, NKO], BF16, kind="ExternalInput").ap()
    wvt = nc.dram_tensor("wvt", [H, NKO], BF16, kind="ExternalInput").ap()
    wot = nc.dram_tensor("wot", [NQO, H], BF16, kind="ExternalInput").ap()
    c2 = nc.dram_tensor("c2", [P, S], BF16, kind="ExternalInput").ap()
    ss = nc.dram_tensor("ss", [P, S], BF16, kind="ExternalInput").ap()
    msk = nc.dram_tensor("msk", [P, P], F32, kind="ExternalInput").ap()
    rot = nc.dram_tensor("rot", [P, P], BF16, kind="ExternalInput").ap()
    y = nc.dram_tensor("y", [S, H], F32, kind="ExternalOutput").ap()

    xt_t = xt.rearrange("(n p) s -> n p s", p=P)
    wqt_t = wqt.rearrange("(n p) o -> n p o", p=P)
    wkt_t = wkt.rearrange("(n p) o -> n p o", p=P)
    wvt_t = wvt.rearrange("(n p) o -> n p o", p=P)
    wot_t = wot.rearrange("(n p) o -> n p o", p=P)
    y_t = y.rearrange("(n p) o -> n p o", p=P)

    with tile.TileContext(nc) as tc, ExitStack() as ctx:
        persist = ctx.enter_context(tc.tile_pool(name="persist", bufs=1))
        xp = ctx.enter_context(tc.tile_pool(name="xp", bufs=32))
        p2 = ctx.enter_context(tc.tile_pool(name="p2", bufs=7))
        p2a = ctx.enter_context(tc.tile_pool(name="p2a", bufs=3))
        p3 = ctx.enter_context(tc.tile_pool(name="p3", bufs=4))
        # all 8 PSUM banks shared across projection + attention + o_proj:
        #   tag "sp"  [128, 1024] x2 (4 banks): score pairs, o_proj, Q/K proj
        #   tag "aux" [128, 512]  x4 (4 banks): AV accum, recip bcast, rope, V
        psum = ctx.enter_context(tc.tile_pool(name="psum", bufs=2, space="PSUM"))

        c2_sb = persist.tile([P, S], BF16, tag="c2", name="c2sb")
        ss_sb = persist.tile([P, S], BF16, tag="ss", name="sssb")
        msk_sb = persist.tile([P, P], F32, tag="msk", name="msksb")
        rot_sb = persist.tile([P, P], BF16, tag="rot", name="rotsb")
        ones65b = persist.tile([65, 64], BF16, tag="ones65b", name="ones65b")

        qtbc = [[persist.tile([P, QC], BF16, tag=f"qtbc{t}_{sc}", name=f"qtbc{t}_{sc}")
                 for sc in range(NSC)] for t in range(4)]
        ktbc = [persist.tile([P, QC], BF16, tag=f"ktbc{sc}", name=f"ktbc{sc}")
                for sc in range(NSC)]
        vaug = [persist.tile([P, 130], BF16, tag=f"vaug{i}", name=f"vaug{i}")
                for i in range(NPT)]
        atbc = [[persist.tile([P, QC], BF16, tag=f"atbc{t}_{qc}", name=f"atbc{t}_{qc}")
                 for qc in range(NSC)] for t in range(4)]
        wotb = [persist.tile([P, S], BF16, tag=f"wotb{t}", name=f"wotb{t}") for t in range(4)]
        wqtb = [persist.tile([P, NQO], BF16, tag=f"wqtb{i}", name=f"wqtb{i}") for i in range(NHT)]
        wktb = [persist.tile([P, NKO], BF16, tag=f"wktb{i}", name=f"wktb{i}") for i in range(NHT)]
        wvtb = [persist.tile([P, NKO], BF16, tag=f"wvtb{i}", name=f"wvtb{i}") for i in range(NHT)]

        # ---- startup: weight + x chunk-0 loads interleaved so the chunk-0
        # Q-proj accumulation (i = 0..15) can start as soon as tile 0 lands.
        xtbc = [[None] * NSC for _ in range(NHT)]
        for i in range(NHT):
            e1 = nc.sync if i % 2 == 0 else nc.gpsimd
            e2 = nc.gpsimd if i % 2 == 0 else nc.sync
            e1.dma_start(wqtb[i][:], wqt_t[i])
            e2.dma_start(wktb[i][:], wkt_t[i])
            e1.dma_start(wvtb[i][:], wvt_t[i])
            xb = xp.tile([P, QC], BF16, tag="xtbc", name=f"xtbc{i}_0")
            e2.dma_start(xb[:], xt_t[i][:, 0:QC])
            xtbc[i][0] = xb
        nc.sync.dma_start(rot_sb[:], rot[:])
        nc.gpsimd.dma_start(msk_sb[:], msk[:])
        nc.sync.dma_start(c2_sb[:], c2[:])
        nc.gpsimd.dma_start(ss_sb[:], ss[:])
        nc.gpsimd.memset(ones65b[64:65, :], 1.0)
        for t in range(4):
            deng = nc.sync if t % 2 == 0 else nc.gpsimd
            deng.dma_start(wotb[t][:], wot_t[t])

        def rope_tile(dst_ap, ps, sc):
            """RoPE: dst = raw*C2 + (R @ raw)*SS for one [128, 512] chunk."""
            ssl = slice(QC * sc, QC * (sc + 1))
            raw = p2a.tile([P, QC], BF16, tag="rope_raw")
            nc.scalar.copy(raw[:], ps[:])
            rps = psum.tile([P, QC], F32, tag="aux", bufs=4, name="rps")
            nc.tensor.matmul(rps[:], lhsT=rot_sb[:], rhs=raw[:],
                             start=True, stop=True)
            t1 = p2a.tile([P, QC], F32, tag="rope_t1")
            nc.vector.tensor_mul(t1[:], raw[:], c2_sb[:, ssl])
            t2 = p2a.tile([P, QC], F32, tag="rope_t2")
            nc.vector.tensor_mul(t2[:], rps[:], ss_sb[:, ssl])
            nc.vector.tensor_add(dst_ap, t1[:], t2[:])

        def attn_step(hp, qc, ki, avpA, avpB, last):
            """Both heads of a pair share one 2-bank score tile; exp always
            runs as ONE ACT instruction (strided 3D AP on diagonal tiles)."""
            j = ki - 4 * qc
            col0 = P * j if j >= 0 else 0
            kc = P * (ki % 4)
            sp = psum.tile([P, 2 * QC], F32, tag="sp", bufs=2, name="sp")
            nc.tensor.matmul(
                sp[:, col0:QC],
                lhsT=ktbc[ki // 4][0:64, kc:kc + P],
                rhs=qtbc[hp][qc][0:64, col0:QC],
                start=True, stop=True,
            )
            nc.tensor.matmul(
                sp[:, QC + col0:2 * QC],
                lhsT=ktbc[ki // 4][64:128, kc:kc + P],
                rhs=qtbc[hp][qc][64:128, col0:QC],
                start=True, stop=True,
            )
            ep = p2.tile([P, 2 * QC], BF16, tag="ep")
            if j >= 0:
                nc.vector.tensor_add(sp[:, col0:col0 + P],
                                     sp[:, col0:col0 + P], msk_sb[:])
                nc.vector.tensor_add(sp[:, QC + col0:QC + col0 + P],
                                     sp[:, QC + col0:QC + col0 + P], msk_sb[:])
                nc.scalar.activation(ep[:, col0:QC], sp[:, col0:QC],
                                     AF.Exp, scale=EXP_SCALE)
                nc.scalar.activation(ep[:, QC + col0:2 * QC],
                                     sp[:, QC + col0:2 * QC],
                                     AF.Exp, scale=EXP_SCALE)
            else:
                nc.scalar.activation(ep[:], sp[:], AF.Exp, scale=EXP_SCALE)
            nc.tensor.matmul(
                avpA[:, col0:QC], lhsT=vaug[ki][:, 0:65],
                rhs=ep[:, col0:QC],
                start=(ki == 0), stop=last,
            )
            nc.tensor.matmul(
                avpB[:, col0:QC], lhsT=vaug[ki][:, 65:130],
                rhs=ep[:, QC + col0:2 * QC],
                start=(ki == 0), stop=last,
            )

        # Softmax normalization, two stages. Stage 1 (per head-half, right
        # after its AV accumulation): copy the [65, QC] PSUM to SBUF and
        # async-DMA the rowsum row onto partition idx of the chunk's rs8
        # gather tile. Stage 2 (once per chunk, deferred to the next chunk):
        # ONE batched [8, QC] DVE reciprocal (8 lanes in parallel -- a
        # [1, QC] reciprocal is serial on a single lane at ~6.5 ns/elem),
        # then per half a K=1 broadcast matmul + normalize multiply. Keeps
        # the ACT stream pure Exp and the DVE stream free of 3.3us recips.
        rs8 = [persist.tile([8, QC], F32, tag=f"rs8_{qc}", name=f"rs8_{qc}")
               for qc in range(NSC)]
        atrs_saved = {}

        def norm_stage1(hp, half, qc, avp):
            idx = 2 * hp + half
            atrs = p2a.tile([65, QC], F32, tag="atrs", bufs=12,
                            name=f"atrs{qc}_{idx}")
            nc.vector.tensor_copy(atrs[:], avp[0:65, :])
            deng = nc.sync if idx % 2 == 0 else nc.gpsimd
            deng.dma_start(rs8[qc][idx:idx + 1, :], atrs[64:65, :])
            atrs_saved[(qc, idx)] = atrs

        def norm_stage2(qc):
            rcp8 = p2a.tile([8, QC], BF16, tag="rcp8", bufs=2)
            with nc.allow_low_precision(reason="softmax denom bcast in bf16"):
                nc.vector.reciprocal(rcp8[:], rs8[qc][:])
            # scatter each reciprocal row back to partition 64 (async DMA)
            # so the K=1 broadcast matmul gets a legal base partition.
            rcpsc = p2a.tile([65, 8 * QC], BF16, tag="rcpsc", bufs=1)
            for idx in range(8):
                deng = nc.sync if idx % 2 == 0 else nc.gpsimd
                deng.dma_start(rcpsc[64:65, idx * QC:(idx + 1) * QC],
                               rcp8[idx:idx + 1, :])
            for hp in range(4):
                for half in range(2):
                    idx = 2 * hp + half
                    rbc = psum.tile([64, QC], F32, tag="aux", bufs=4,
                                    name="rbc")
                    nc.tensor.matmul(rbc[:], lhsT=ones65b[64:65, 0:64],
                                     rhs=rcpsc[64:65, idx * QC:(idx + 1) * QC],
                                     start=True, stop=True)
                    nc.vector.tensor_mul(
                        atbc[hp][qc][64 * half:64 * half + 64, :],
                        atrs_saved[(qc, idx)][0:64, :], rbc[:])

        def oproj_piece(qc, st):
            stj = st - 4 * qc
            for oc in range(NSC):
                op = psum.tile([P, QC], F32, tag="sp", bufs=2, name="op")
                for ft in range(4):
                    nc.tensor.matmul(
                        op[:],
                        lhsT=atbc[ft][qc][:, P * stj:P * (stj + 1)],
                        rhs=wotb[ft][:, QC * oc:QC * (oc + 1)],
                        start=(ft == 0), stop=(ft == 3),
                    )
                ost = p3.tile([P, QC], F32, tag="ost")
                nc.vector.tensor_copy(ost[:], op[:])
                nc.sync.dma_start(y_t[st][:, QC * oc:QC * (oc + 1)], ost[:])

        from collections import deque
        oproj_q = deque()
        for sc in range(NSC):
            # Q^T chunks
            for t in range(4):
                ps = psum.tile([P, QC], F32, tag="sp", bufs=2, name="qkps")
                for i in range(NHT):
                    nc.tensor.matmul(
                        ps[:], lhsT=wqtb[i][:, P * t:P * (t + 1)],
                        rhs=xtbc[i][sc][:],
                        start=(i == 0), stop=(i == NHT - 1),
                    )
                rope_tile(qtbc[t][sc][:], ps, sc)
            # K^T chunk
            ps = psum.tile([P, QC], F32, tag="sp", bufs=2, name="qkps")
            for i in range(NHT):
                nc.tensor.matmul(
                    ps[:], lhsT=wktb[i][:], rhs=xtbc[i][sc][:],
                    start=(i == 0), stop=(i == NHT - 1),
                )
            rope_tile(ktbc[sc][:], ps, sc)
            # V tiles in this chunk
            for j in range(4 * sc, 4 * sc + 4):
                jj = j - 4 * sc
                ps = psum.tile([P, NKO], F32, tag="aux", bufs=4, name="vps")
                for i in range(NHT):
                    nc.tensor.matmul(
                        ps[:], lhsT=xtbc[i][sc][:, P * jj:P * (jj + 1)],
                        rhs=wvtb[i][:],
                        start=(i == 0), stop=(i == NHT - 1),
                    )
                nc.vector.tensor_copy(vaug[j][:, 0:64], ps[:, 0:64])
                nc.vector.tensor_copy(vaug[j][:, 65:129], ps[:, 64:128])
                nc.gpsimd.memset(vaug[j][:, 64:65], 1.0)
                nc.gpsimd.memset(vaug[j][:, 129:130], 1.0)

            # prefetch next chunk's X columns (overlaps with attention)
            if sc + 1 < NSC:
                for i in range(NHT):
                    deng = nc.sync if i % 2 == 0 else nc.gpsimd
                    xb = xp.tile([P, QC], BF16, tag="xtbc",
                                 name=f"xtbc{i}_{sc + 1}")
                    deng.dma_start(xb[:], xt_t[i][:, QC * (sc + 1):QC * (sc + 2)])
                    xtbc[i][sc + 1] = xb

            # ---- attention for qc = sc (causal: only needs chunks <= sc) --
            qc = sc
            nkt = 4 * qc + 4
            for hp in range(4):
                if hp == 0 and sc > 0:
                    norm_stage2(sc - 1)
                avpA = psum.tile([65, QC], F32, tag="aux", bufs=4, name="avpA")
                avpB = psum.tile([65, QC], F32, tag="aux", bufs=4, name="avpB")
                for ki in range(nkt):
                    attn_step(hp, qc, ki, avpA, avpB, ki == nkt - 1)
                norm_stage1(hp, 0, qc, avpA)
                norm_stage1(hp, 1, qc, avpB)
                # deferred o_proj pieces fill PE while exps/AVs drain
                if hp >= 1 and oproj_q:
                    oproj_piece(*oproj_q.popleft())
                if hp == 3 and oproj_q:
                    oproj_piece(*oproj_q.popleft())
            for st in range(4 * qc, 4 * qc + 4):
                oproj_q.append((qc, st))
        norm_stage2(NSC - 1)
        while oproj_q:
            oproj_piece(*oproj_q.popleft())

    nc.compile()
    return nc


def _host_tables():
    import ml_dtypes
    BF = ml_dtypes.bfloat16
    inv_freq = 1.0 / (10000.0 ** (np.arange(0, HD, 2, dtype=np.float32) / HD))
    pos = np.arange(S, dtype=np.float32)
    freqs = np.einsum('s,d->sd', pos, inv_freq)          # [S, 32]
    emb = np.concatenate([freqs, freqs], axis=-1)        # [S, 64]
    cosT = np.cos(emb).T.astype(np.float32)              # [64, S]
    sinT = np.sin(emb).T.astype(np.float32)
    c2 = np.ascontiguousarray(np.vstack([cosT, cosT])).astype(BF)   # [128, S]
    # sign of rotate_half is encoded in the rot matrix below; ss is plain sin
    ss = np.ascontiguousarray(np.vstack([sinT, sinT])).astype(BF)
    # rotate-half as a matmul: out[d] = sum_d' R[d', d] * in[d']
    R64 = np.zeros((HD, HD), dtype=np.float32)
    for d in range(32):
        R64[d + 32, d] = -1.0       # out[d] = -in[d+32]
        R64[d, d + 32] = 1.0        # out[d+32] = in[d]
    rot = np.zeros((P, P), dtype=np.float32)
    rot[0:64, 0:64] = R64
    rot[64:128, 64:128] = R64
    # causal bias for a diagonal 128x128 tile in scores^T[k, q] layout
    kk = np.arange(P)[:, None]
    qq = np.arange(P)[None, :]
    msk = np.where(kk <= qq, 0.0, MASK_VAL).astype(np.float32)
    rot = rot.astype(BF)   # exact: entries are 0/+-1
    return c2, ss, rot, msk


# q/o head order within a rank block: pair heads (u, u+4) in each 128-row tile
_HEAD_ORDER = [0, 4, 1, 5, 2, 6, 3, 7]


def _make_in_maps(hidden_states, Wq, Wk, Wv, Wo):
    import ml_dtypes
    BF = ml_dtypes.bfloat16
    hs = np.asarray(hidden_states, dtype=np.float32)
    Wq = np.asarray(Wq, dtype=np.float32)
    Wk = np.asarray(Wk, dtype=np.float32)
    Wv = np.asarray(Wv, dtype=np.float32)
    Wo = np.asarray(Wo, dtype=np.float32)
    c2, ss, rot, msk = _host_tables()
    in_maps = []
    for c in range(8):
        b, r = c // 4, c % 4
        # row indices of Wq (= cols of Wo) for this rank, in device head order
        qrows = np.concatenate([
            np.arange(HD) + (NHL * r + u) * HD for u in _HEAD_ORDER
        ])
        in_maps.append({
            "xt": np.ascontiguousarray(hs[b].T).astype(BF),
            "wqt": np.ascontiguousarray(Wq[qrows, :].T).astype(BF),
            "wkt": np.ascontiguousarray(Wk[NKO * r:NKO * (r + 1), :].T).astype(BF),
            "wvt": np.ascontiguousarray(Wv[NKO * r:NKO * (r + 1), :].T).astype(BF),
            "wot": np.ascontiguousarray(Wo[:, qrows].T).astype(BF),
            "c2": c2, "ss": ss, "msk": msk, "rot": rot,
        })
    return in_maps


_NC = None


def _get_nc():
    global _NC
    if _NC is None:
        _NC = build_nc()
    return _NC


def run_cores(hidden_states, Wq, Wk, Wv, Wo, **run_kwargs):
    """Run the SPMD kernel; returns (out [B,S,H] fp32, BassKernelResults)."""
    nc = _get_nc()
    in_maps = _make_in_maps(hidden_states, Wq, Wk, Wv, Wo)
    res = run_bass_kernel_spmd(nc, in_maps, list(range(8)), **run_kwargs)
    out = np.zeros((B, S, H), dtype=np.float32)
    for c in range(8):
        out[c // 4] += res.results[c]["y"]
    return out, res


def kernel(hidden_states, Wq, Wk, Wv, Wo):
    out, _ = run_cores(hidden_states, Wq, Wk, Wv, Wo)
    return out
